# revision 1
# baseline (speedup 1.0000x reference)
"""Trainium2 Bass kernel for causal self-attention (GPT-J RoPE), 8-way
tensor-parallel over heads.

Contract: kernel(x, W_qkv, W_proj) -> np.ndarray  (full [T, D] output)

Sharding: 16 heads / 8 cores = 2 heads per core. Each core computes its
2 heads' QKV projection, RoPE, causal attention, and its partial
W_proj contribution; the host sums the 8 partial outputs (the TP
all-reduce), which is the unshard step.

Per-core device program (all fp32, matmuls in float32r):
  phase B: qT/kT/vT = W @ xT streamed over d-tiles; RoPE via a pair-swap
           permutation matmul + cos/sin elementwise; v transposed to
           [tk, c] via PE and augmented with a ones column.
  phase C: per 512-wide query block i (flash-style, causal):
           ST[tk, tq] score blocks (both heads row-packed, K=64),
           exp on ScalarE straight from PSUM (scale=1/sqrt(C), no max
           subtraction -- scores are ~N(0,1)), diagonal-block masking via
           gpsimd affine_select, AV matmuls with M=65 (v | ones) so the
           softmax denominator falls out of the same pass, then
           reciprocal + partition_broadcast + fused normalize/evict and
           the W_proj matmul (K=64 per head half).
"""

import math
import sys

if "/opt/trn_rl_repo" not in sys.path:
    sys.path.insert(0, "/opt/trn_rl_repo")

import numpy as np

import concourse.bass as bass  # noqa: F401  (engine namespaces live on nc)
import concourse.mybir as mybir
import concourse.tile as tile
from concourse import bacc
from concourse.bass_utils import run_bass_kernel_spmd
from concourse.masks import make_identity

F32 = mybir.dt.float32
F32R = mybir.dt.float32r

N_CORES = 8
N_HEAD = 16
T_FULL = 4096
D_FULL = 1024
C_HEAD = 64


def build_program(T=4096, D=1024, C=64, use_f32r=True, num_devices=8,
                  bufs_pst=2, bufs_po=1, bufs_x=8, bufs_est=6, bufs_out=3,
                  bufs_pp=2, bufs_scr=3, bufs_cs=3, bufs_acc=2, ablate=None):
    """Build the per-core Bass program. C2 = 2 heads * C = 128 partitions."""
    HPC = 2
    C2 = HPC * C
    assert C2 == 128
    TQB = 512                 # query block width
    ND = D // 128             # d-tiles for the QKV contraction
    NT = T // 128             # key tiles
    NI = T // TQB             # query blocks
    JPB = TQB // 128          # key tiles per query block (4)
    NG = max(1, D // TQB)         # proj output chunks
    DW = D // NG                  # proj chunk width (<= 512)
    assert T % TQB == 0 and D % 128 == 0 and DW <= TQB and D % NG == 0

    MMF = F32R if use_f32r else F32   # dtype for matmul-feeding tensors

    def R(ap):
        return ap

    nc = bacc.Bacc(
        "TRN2",
        target_bir_lowering=False,
        debug=False,
        enable_asserts=False,
        num_devices=num_devices,
    )

    xT_d = nc.dram_tensor("xT", [D, T], MMF, kind="ExternalInput").ap()
    wq_d = nc.dram_tensor("wq", [D, C2], MMF, kind="ExternalInput").ap()
    wk_d = nc.dram_tensor("wk", [D, C2], MMF, kind="ExternalInput").ap()
    wv_d = nc.dram_tensor("wv", [D, C2], MMF, kind="ExternalInput").ap()
    perm_d = nc.dram_tensor("perm", [C2, C2], MMF, kind="ExternalInput").ap()
    cos_d = nc.dram_tensor("cosT", [C2, T], F32, kind="ExternalInput").ap()
    sin_d = nc.dram_tensor("sinT", [C2, T], F32, kind="ExternalInput").ap()
    wp0_d = nc.dram_tensor("wp0", [C, D], MMF, kind="ExternalInput").ap()
    wp1_d = nc.dram_tensor("wp1", [C, D], MMF, kind="ExternalInput").ap()
    out_d = nc.dram_tensor("out", [T, D], F32, kind="ExternalOutput").ap()

    scale = 1.0 / math.sqrt(C)

    with tile.TileContext(nc) as tc:
        with (
            tc.tile_pool(name="const", bufs=1) as pconst,
            tc.tile_pool(name="cs", bufs=bufs_cs) as pcs,
            tc.tile_pool(name="xs", bufs=bufs_x) as px,
            tc.tile_pool(name="scr", bufs=bufs_scr) as pscr,
            tc.tile_pool(name="qk", bufs=1) as pqk,
            tc.tile_pool(name="vt", bufs=1) as pv,
            tc.tile_pool(name="est", bufs=bufs_est) as pest,
            tc.tile_pool(name="ot", bufs=1) as pot,
            tc.tile_pool(name="rd", bufs=2) as prd,
            tc.tile_pool(name="outsb", bufs=bufs_out) as pout,
        ):
            # ---- constants ----
            wq_sb, wk_sb, wv_sb = [], [], []
            for d in range(ND):
                for lst, nm, drt in ((wq_sb, "wq", wq_d), (wk_sb, "wk", wk_d),
                                     (wv_sb, "wv", wv_d)):
                    t_ = pconst.tile([128, C2], MMF, tag=f"{nm}{d}", name=f"{nm}{d}")
                    nc.sync.dma_start(t_[:], drt[d * 128:(d + 1) * 128, :])
                    lst.append(t_)
            perm_sb = pconst.tile([C2, C2], MMF, tag="perm", name="perm_sb")
            nc.sync.dma_start(perm_sb[:], perm_d[:])
            ident = pconst.tile([128, 128], F32, tag="ident", name="ident")
            make_identity(nc, ident[:])
            wp0_sb = pconst.tile([C, D], MMF, tag="wp0", name="wp0_sb")
            nc.sync.dma_start(wp0_sb[:], wp0_d[:])
            wp1_sb = pconst.tile([C, D], MMF, tag="wp1", name="wp1_sb")
            nc.sync.dma_start(wp1_sb[:], wp1_d[:])

            # persistent rope'd q/k chunks and v tiles
            qr_t = [pqk.tile([C2, TQB], MMF, tag=f"qr{i}", name=f"qr{i}")
                    for i in range(NI)]
            kr_t = [pqk.tile([C2, TQB], MMF, tag=f"kr{i}", name=f"kr{i}")
                    for i in range(NI)]
            # v tiles [tk, c|1] per head, ones column at col C
            v_t = [[pv.tile([128, C + 1], MMF, tag=f"v{h}_{j}", name=f"v{h}_{j}")
                    for j in range(NT)] for h in range(HPC)]
            ones_col = pconst.tile([128, 1], F32, tag="ones", name="ones_col")
            nc.vector.memset(ones_col[:], 1.0)
            for h in range(HPC):
                for j in range(NT):
                    nc.vector.tensor_copy(v_t[h][j][:, C:C + 1], ones_col[:])
            ot_t = [[pot.tile([C, TQB], MMF, tag=f"ot{h}_{i}", name=f"ot{h}_{i}")
                     for i in range(NI)] for h in range(HPC)]

            # ================= phase B: qkv + rope + v transpose ============
            with (
                tc.tile_pool(name="bacc", bufs=bufs_acc, space="PSUM") as pacc,
                tc.tile_pool(name="brot", bufs=1, space="PSUM") as prot_p,
                tc.tile_pool(name="bvt", bufs=1, space="PSUM") as pvt_p,
            ):
                for i in range(NI):
                    cosc = pcs.tile([C2, TQB], F32, tag="cos", name="cosc")
                    nc.sync.dma_start(cosc[:], cos_d[:, i * TQB:(i + 1) * TQB])
                    sinc = pcs.tile([C2, TQB], F32, tag="sin", name="sinc")
                    nc.sync.dma_start(sinc[:], sin_d[:, i * TQB:(i + 1) * TQB])

                    pq = pacc.tile([C2, TQB], F32, tag="pq", name="pq")
                    pk = pacc.tile([C2, TQB], F32, tag="pk", name="pk")
                    pvp = pacc.tile([C2, TQB], F32, tag="pv", name="pvp")
                    for d in range(ND):
                        xt = px.tile([128, TQB], MMF, tag="xt", name="xt")
                        nc.sync.dma_start(
                            xt[:], xT_d[d * 128:(d + 1) * 128,
                                        i * TQB:(i + 1) * TQB])
                        st, sp = (d == 0), (d == ND - 1)
                        nc.tensor.matmul(pq[:], R(wq_sb[d][:]), R(xt[:]),
                                         start=st, stop=sp)
                        nc.tensor.matmul(pk[:], R(wk_sb[d][:]), R(xt[:]),
                                         start=st, stop=sp)
                        nc.tensor.matmul(pvp[:], R(wv_sb[d][:]), R(xt[:]),
                                         start=st, stop=sp)

                    # rope on q and k
                    for psrc, dst in ((pq, qr_t[i]), (pk, kr_t[i])):
                        raw = pscr.tile([C2, TQB], MMF, tag="raw", name="raw")
                        nc.vector.tensor_copy(raw[:], psrc[:])
                        prot = prot_p.tile([C2, TQB], F32, tag="rot", name="prot")
                        nc.tensor.matmul(prot[:], R(perm_sb[:]), R(raw[:]),
                                         start=True, stop=True)
                        qc = pscr.tile([C2, TQB], F32, tag="qc", name="qc")
                        nc.vector.tensor_mul(qc[:], psrc[:], cosc[:])
                        qs = pscr.tile([C2, TQB], F32, tag="qs", name="qs")
                        nc.vector.tensor_mul(qs[:], prot[:], sinc[:])
                        nc.vector.tensor_add(dst[:], qc[:], qs[:])

                    # v: evict + transpose to [tk, c] per head
                    vraw = pscr.tile([C2, TQB], F32, tag="vraw", name="vraw")
                    nc.vector.tensor_copy(vraw[:], pvp[:])
                    for s in range(JPB):
                        j = i * JPB + s
                        pvt = pvt_p.tile([128, 128], F32, tag="pvt", name="pvt")
                        nc.tensor.transpose(pvt[:], vraw[:, s * 128:(s + 1) * 128],
                                            ident[:])
                        nc.vector.tensor_copy(v_t[0][j][:, 0:C], pvt[:, 0:C])
                        nc.vector.tensor_copy(v_t[1][j][:, 0:C], pvt[:, C:C2])

            # ================= phase C: attention + proj ====================
            with (
                tc.tile_pool(name="pst", bufs=bufs_pst, space="PSUM") as pst_p,
                tc.tile_pool(name="po", bufs=bufs_po, space="PSUM") as po_p,
                tc.tile_pool(name="pp", bufs=bufs_pp, space="PSUM") as pp_p,
            ):
                for i in range(NI if ablate != "B" else 0):
                    po = po_p.tile([128, 2 * TQB], F32, tag="po", name="po")
                    njt = (i + 1) * JPB
                    for jg in range(0, njt, 2):
                        js = [j for j in (jg, jg + 1) if j < njt]
                        los = [max(TQB * i, 128 * j) for j in js]
                        ws = [TQB * (i + 1) - lo for lo in los]
                        offs = list(np.cumsum([0] + ws[:-1]))
                        wtot = int(sum(ws))
                        psts, ests = [], []
                        for h in range(HPC):
                            psts.append(pst_p.tile([128, 2 * TQB], F32,
                                                   tag="pst", name="pst"))
                            ests.append(pest.tile([128, 2 * TQB], MMF,
                                                  tag="est", name="est"))
                        # scores (row-packed across heads)
                        for j, lo, w, o in zip(js, los, ws, offs):
                            jc, jo = divmod(j, JPB)
                            for h in range(HPC):
                                klhs = kr_t[jc][h * C:(h + 1) * C,
                                                jo * 128:(jo + 1) * 128]
                                qrhs = qr_t[i][h * C:(h + 1) * C,
                                               lo - TQB * i:lo - TQB * i + w]
                                nc.tensor.matmul(psts[h][:, o:o + w],
                                                 R(klhs), R(qrhs),
                                                 start=True, stop=True)
                        for h in range(HPC):
                            nc.scalar.activation(ests[h][:, 0:wtot],
                                                 psts[h][:, 0:wtot],
                                                 mybir.ActivationFunctionType.Exp,
                                                 scale=scale)
                            for j, lo, w, o in zip(js, los, ws, offs):
                                if 128 * j >= TQB * i:  # diagonal block
                                    nc.gpsimd.affine_select(
                                        out=ests[h][:, o:o + 128],
                                        in_=ests[h][:, o:o + 128],
                                        compare_op=mybir.AluOpType.is_ge,
                                        fill=0.0, base=0,
                                        pattern=[[1, 128]],
                                        channel_multiplier=-1)
                        # AV with ones column -> O and denominator
                        for j, lo, w, o in zip(js, los, ws, offs):
                            for h in range(HPC):
                                cb = h * TQB + (lo - TQB * i)
                                nc.tensor.matmul(
                                    po[0:C + 1, cb:cb + w],
                                    R(v_t[h][j][:]), R(ests[h][:, o:o + w]),
                                    start=(j == 0), stop=(j == njt - 1),
                                    skip_group_check=True)
                    # normalize: O / denom
                    for h in range(HPC if ablate not in ("AV",) else 0):
                        rd = prd.tile([1, TQB], F32, tag="rd", name="rd")
                        nc.vector.reciprocal(rd[:],
                                             po[C:C + 1, h * TQB:(h + 1) * TQB])
                        rdb = prd.tile([C, TQB], F32, tag="rdb", name="rdb")
                        nc.gpsimd.partition_broadcast(rdb[:], rd[:])
                        nc.vector.tensor_mul(ot_t[h][i][:],
                                             po[0:C, h * TQB:(h + 1) * TQB],
                                             rdb[:])
                    # projection for this block's 4 row-tiles
                    for s in range(JPB if ablate not in ("AV", "NORM") else 0):
                        osb = pout.tile([128, D], F32, tag="osb", name="osb")
                        for g in range(NG):
                            pp = pp_p.tile([128, DW], F32, tag="pp", name="pp")
                            nc.tensor.matmul(
                                pp[:], R(ot_t[0][i][:, s * 128:(s + 1) * 128]),
                                R(wp0_sb[:, g * DW:(g + 1) * DW]),
                                start=True, stop=False)
                            nc.tensor.matmul(
                                pp[:], R(ot_t[1][i][:, s * 128:(s + 1) * 128]),
                                R(wp1_sb[:, g * DW:(g + 1) * DW]),
                                start=False, stop=True)
                            nc.vector.tensor_copy(osb[:, g * DW:(g + 1) * DW],
                                                  pp[:])
                        tt = i * JPB + s
                        nc.sync.dma_start(out_d[tt * 128:(tt + 1) * 128, :],
                                          osb[:])

    nc.compile()
    return nc


def host_inputs(x, W_qkv, W_proj, n_cores=N_CORES):
    """Shard full inputs into per-core input maps."""
    x = np.asarray(x, np.float32)
    W_qkv = np.asarray(W_qkv, np.float32)
    W_proj = np.asarray(W_proj, np.float32)
    T, D = x.shape
    C = C_HEAD
    HPC = (3 * D // 3) // C // n_cores  # heads per core
    H = D // C
    HPC = H // n_cores
    C2 = HPC * C
    Wq, Wk, Wv = W_qkv[0:D], W_qkv[D:2 * D], W_qkv[2 * D:3 * D]

    xT = np.ascontiguousarray(x.T)

    # rope tables [C2, T]
    inv_freq = 1.0 / (10000.0 ** (np.arange(0, C, 2, dtype=np.float64) / C))
    ang = np.arange(T, dtype=np.float64)[None, :] * \
        np.repeat(inv_freq, 2)[:, None]          # [C, T]
    cosT = np.tile(np.cos(ang), (HPC, 1)).astype(np.float32)
    sinT = np.tile(np.sin(ang), (HPC, 1)).astype(np.float32)
    cosT = np.ascontiguousarray(cosT)
    sinT = np.ascontiguousarray(sinT)

    # pair-swap-negate permutation: rot = perm.T @ q  (within each head block)
    perm = np.zeros((C2, C2), np.float32)
    for cp in range(C2):
        if cp % 2 == 0:
            perm[cp + 1, cp] = -1.0
        else:
            perm[cp - 1, cp] = 1.0

    in_maps = []
    for c in range(n_cores):
        rows = slice(c * C2, (c + 1) * C2)
        in_maps.append({
            "xT": xT,
            "wq": np.ascontiguousarray(Wq[rows].T),
            "wk": np.ascontiguousarray(Wk[rows].T),
            "wv": np.ascontiguousarray(Wv[rows].T),
            "perm": perm,
            "cosT": cosT,
            "sinT": sinT,
            "wp0": np.ascontiguousarray(W_proj[:, c * C2:c * C2 + C].T),
            "wp1": np.ascontiguousarray(W_proj[:, c * C2 + C:(c + 1) * C2].T),
        })
    return in_maps


_PROGRAM_CACHE = {}


def _get_program(T, D, use_f32r=True):
    key = (T, D, use_f32r)
    if key not in _PROGRAM_CACHE:
        _PROGRAM_CACHE[key] = build_program(T=T, D=D, use_f32r=use_f32r)
    return _PROGRAM_CACHE[key]


def run_cores(x, W_qkv, W_proj, use_f32r=True, **run_kwargs):
    """Run the SPMD program on 8 cores, return BassKernelResults."""
    nc = _get_program(x.shape[0], x.shape[1], use_f32r)
    in_maps = host_inputs(x, W_qkv, W_proj)
    return run_bass_kernel_spmd(nc, in_maps, core_ids=list(range(N_CORES)),
                                **run_kwargs)


def kernel(x, W_qkv, W_proj):
    res = run_cores(x, W_qkv, W_proj)
    out = np.zeros((x.shape[0], x.shape[1]), np.float32)
    for r in res.results:
        out += r["out"]
    return out

